# revision 1
# baseline (speedup 1.0000x reference)
"""Trainium2 Bass kernel for nn_AttentionMemory (sparse_attention).

reference:
    mkf = mk.reshape(B, CK, HW); qkf = qk.reshape(B, CK, HW)
    affinity[b, m, q] = (-|mk_m|^2 + 2 mk_m.qk_q - |qk_q|^2) / sqrt(CK)
    out = softmax(affinity, axis=m)

Math used here: softmax over m is invariant to any additive term that is
constant in m, so the -|qk_q|^2 term (and any global constant C) drops out:
    out[b, :, q] = softmax_m( mk_m.qk_q/4 - (|mk_m|^2 - C)/8 )
with sqrt(CK) = 8, C = 64 (centers the exponent near 0).

Distribution: pure data parallelism over B=16 -> 2 batches per core on 8
NeuronCores. Each core computes out[b, :, :] for its 2 local batches.

Per-core layout (m on partitions, q on the free axis so the output DMA is
contiguous):
  - Both local batches are stacked on partitions: mk4_all[128, HW] rows
    0..63 = batch0 mk/4, rows 64..127 = batch1 mk/4 (fp32r-rounded); same
    for qk_all (unscaled qk).
  - main matmul (fp32r, 1 cyc/row): lhsT = mk4_all[b*64:, m-tile] (K=64,
    stationary), rhs = qk_all[b*64:, q-chunk] -> PSUM z[m_tile, q]
  - ScalarE exp with per-partition bias = -(a[m]-64)/8 (a = |mk_m|^2),
    which is exactly the softmax m-term -> E in SBUF
  - column sums s[q] via an all-ones[128,128] lhsT matmul accumulated over
    the 18 m-tiles (the result lands broadcast to every partition for free)
  - reciprocal_approx_fast, VectorE multiply, contiguous DMA out.
"""

import numpy as np
from contextlib import ExitStack

import concourse.bass as bass
import concourse.tile as tile
from concourse import bacc, mybir
from concourse.bass_utils import run_bass_kernel_spmd

B, CK, H, W = 16, 64, 36, 64
HW = H * W                 # 2304
NCORES = 8
BL = B // NCORES           # 2 batches per core
MT = HW // 128             # 18 m-tiles
QCH = 768                  # q chunk (2 PSUM banks)
NQ = HW // QCH             # 3
F32 = mybir.dt.float32
F32R = mybir.dt.float32r
BF16 = mybir.dt.bfloat16
AF = mybir.ActivationFunctionType
ALU = mybir.AluOpType

# matmul free-dim chunks inside one 768-wide q chunk (PSUM bank = 512 f32)
MM_CHUNKS = ((0, 512), (512, 256))


def _build_kernel(tc: tile.TileContext, out_ext, mk_ext, qk_ext):
    nc = tc.nc
    ones_dram = nc.inline_tensor(np.ones((128, 128), np.float32))
    with ExitStack() as ctx:
        singles = ctx.enter_context(tc.tile_pool(name="singles", bufs=1))
        mk4_all = singles.tile([128, HW], F32R, name="mk4_all")
        qk_all = singles.tile([128, HW], F32R, name="qk_all")
        ones_f = singles.tile([128, 128], F32, name="ones_f")
        ones_r = singles.tile([128, 128], F32R, name="ones_r")
        # bias columns: neg_a8[p, b*MT+t] = -(a[b, t*128+p] - 64)/8
        neg_a8 = singles.tile([128, BL * MT], F32, name="neg_a8")
        # warmup-chunk (b=0) multiplicative form of the bias: w = exp(neg_a8),
        # and rank-1 lhsT matrices W_r[:, t, j] = w[:, t] for weighted sums
        w_sb = singles.tile([128, MT], F32, name="w_sb")
        W_r = singles.tile([128, MT, 128], F32R, name="W_r")
        # tiny dummy exp so the ACT table set loads at t~0, not before the
        # first real Activation
        dummy = singles.tile([1, 1], F32, name="dummy")

        prep = ctx.enter_context(tc.tile_pool(name="prep", bufs=1))
        e_sb_pool = ctx.enter_context(tc.tile_pool(name="e_sb", bufs=2))
        o_pool = ctx.enter_context(tc.tile_pool(name="o_sb", bufs=10))
        r_pool = ctx.enter_context(tc.tile_pool(name="r", bufs=4))
        psum_e = ctx.enter_context(tc.tile_pool(name="psum_e", bufs=3, space="PSUM"))
        psum_s = ctx.enter_context(tc.tile_pool(name="psum_s", bufs=1, space="PSUM"))

        # ---- prep (both batches at once; they're stacked on partitions) ----
        mk_raw = prep.tile([128, HW], F32, name="mk_raw")
        qk_raw = prep.tile([128, HW], F32, name="qk_raw")
        # DMA order = first-needed first: mk (sq/bias chain), the raw first
        # q-chunk, the ones block, then the rest of qk
        qk_flat = qk_ext.rearrange("b c n -> (b c) n")
        mk_flat = mk_ext.rearrange("b c n -> (b c) n")
        HH = HW // 2
        nc.sync.dma_start(mk_raw[:, 0:HH], mk_flat[:, 0:HH])
        nc.sync.dma_start(qk_raw[:, 0:256], qk_flat[:, 0:256])
        nc.sync.dma_start(ones_f[:], ones_dram.ap())
        nc.sync.dma_start(mk_raw[:, HH:HW], mk_flat[:, HH:HW])
        nc.sync.dma_start(qk_raw[:, 256:HW], qk_flat[:, 256:HW])
        # tiny dummy exp on the first-landing tensor so the ACT table set
        # loads during the input DMAs, not before the first real Activation
        nc.scalar.activation(dummy[:], mk_raw[0:1, 0:1], AF.Exp)
        nc.gpsimd.tensor_copy(ones_r[:], ones_f[:])
        # sq = (mk/4)^2 = mk^2/16 gates the exp bias -> first on DVE. The
        # rounded-to-f32r matmul copies (BIR verifier requirement) go to
        # GpSimd; chunk 0 runs raw so they're only needed from chunk 1 on.
        sq = prep.tile([128, HW], F32, name="sq")
        # fast mini-prep for the warmup chunk: its rhs (first 256 q cols) and
        # lhsT (mk4) convert on DVE right behind the DMA pieces; the bulk of
        # qk converts on GpSimd in parallel
        nc.vector.tensor_copy(qk_all[:, 0:256], qk_raw[:, 0:256])
        nc.gpsimd.tensor_copy(qk_all[:, 256:HW], qk_raw[:, 256:HW])
        # the sq -> a -> bias chain runs in m-halves so the first chunk's
        # exps only wait on the first half of the mk transfer
        a_ps = psum_s.tile([128, QCH], F32, name="s_ps")[:, 0 : BL * MT]
        a_3d = a_ps.rearrange("p (u t) -> p u t", u=BL)
        n_3d = neg_a8.rearrange("p (u t) -> p u t", u=BL)
        for lo, hi in ((0, MT // 2), (MT // 2, MT)):
            nc.vector.tensor_scalar_mul(
                mk4_all[:, lo * 128 : hi * 128], mk_raw[:, lo * 128 : hi * 128], 0.25
            )
            nc.vector.scalar_tensor_tensor(
                out=sq[:, lo * 128 : hi * 128],
                in0=mk_raw[:, lo * 128 : hi * 128],
                scalar=0.0625,
                in1=mk_raw[:, lo * 128 : hi * 128],
                op0=ALU.mult,
                op1=ALU.mult,
            )
            for t in range(lo, hi):
                for b in range(BL):
                    nc.tensor.matmul(
                        a_ps[:, b * MT + t : b * MT + t + 1],
                        lhsT=sq[b * CK : (b + 1) * CK, t * 128 : (t + 1) * 128],
                        rhs=ones_f[b * CK : (b + 1) * CK, 0:1],
                        start=True,
                        stop=True,
                    )
            nc.vector.tensor_scalar(
                out=n_3d[:, :, lo:hi],
                in0=a_3d[:, :, lo:hi],
                scalar1=-2.0,
                scalar2=8.0,
                op0=ALU.mult,
                op1=ALU.add,
            )

        def emit_warmup_chunk(q0, w):
            # b=0 only. exps run bias-free so THREE m-tiles merge into one
            # Activation (one e_ps tile holds 3 matmul outputs); the missing
            # exp(bias) factor w[m] is applied in the sum-matmul lhsT (W_r)
            # and in the store multiply (folded into the replicated r).
            G = 3
            e_sb = e_sb_pool.tile([128, MT, QCH], F32R, name="e_sb")
            s_ps = psum_s.tile([128, QCH], F32, name="s_ps")

            def emit_w_half(lo, hi):
                # placed between merged exps so the in-order ACT queue isn't
                # blocked waiting for the sq -> a -> neg_a8 chain
                nc.scalar.activation(w_sb[:, lo:hi], neg_a8[:, lo:hi], AF.Exp)
                for t in range(lo, hi):
                    nc.vector.tensor_scalar_mul(
                        W_r[:, t, :], ones_f[:], w_sb[:, t : t + 1]
                    )

            def sum_mm(t):
                nc.tensor.matmul(
                    s_ps[:, 0:w],
                    lhsT=W_r[:, t, :],
                    rhs=e_sb[:, t, 0:w],
                    start=(t == 0),
                    stop=(t == MT - 1),
                )

            for g in range(MT // G):
                e_ps = psum_e.tile([128, QCH], F32, name="e_ps")
                for i in range(G):
                    t = g * G + i
                    nc.tensor.matmul(
                        e_ps[:, i * w : (i + 1) * w],
                        lhsT=mk4_all[0:CK, t * 128 : (t + 1) * 128],
                        rhs=qk_all[0:CK, q0 : q0 + w],
                        start=True,
                        stop=True,
                    )
                nc.scalar.activation(
                    e_sb[:, g * G : (g + 1) * G, 0:w],
                    e_ps[:, 0 : G * w],
                    AF.Exp,
                )
                if g == 0:
                    emit_w_half(0, MT // 2)
                elif g == 1:
                    emit_w_half(MT // 2, MT)
                if g >= 1:
                    for i in range(G):
                        sum_mm((g - 1) * G + i)
            for i in range(G):
                sum_mm(MT - G + i)
            r_sb = r_pool.tile([128, QCH], F32, name="r_sb")
            nc.vector.reciprocal_approx_fast(r_sb[:, 0:w], s_ps[:, 0:w])
            # paired stores; wr2[:, u, :] = w[:, t+u] * r folds the weight in
            for t2 in range(MT // 2):
                t = 2 * t2
                wr2 = r_pool.tile([128, QCH], F32, name="r_sb")
                for u in range(2):
                    nc.vector.tensor_scalar_mul(
                        wr2[:, u * w : (u + 1) * w],
                        r_sb[:, 0:w],
                        w_sb[:, t + u : t + u + 1],
                    )
                wv = wr2[:, 0 : 2 * w].rearrange("p (u q) -> p u q", u=2)
                o_sb = o_pool.tile([128, QCH], F32, name="o_sb")
                ov = o_sb[:, 0 : 2 * w].rearrange("p (u q) -> p u q", u=2)
                nc.vector.tensor_mul(
                    ov, e_sb[:, t : t + 2, 0:w].bitcast(F32), wv
                )
                dest = out_ext[
                    0, t * 128 : (t + 2) * 128, q0 : q0 + w
                ].rearrange("(u p) q -> p u q", u=2)
                nc.sync.dma_start(dest, ov)

        # ---- main: one softmax pipeline per (batch, q-chunk) ----
        def emit_chunk(b, q0, w, raw=False):
            # raw=True: run on the un-prepped f32 DMA data (the /4 scale is
            # folded into the exp's scale operand) so the first chunk has no
            # dependency on the f32r prep passes; fp32 matmul is 4 cyc/row
            # but N=256 keeps that off the critical path.
            lhs_all = mk_raw if raw else mk4_all
            rhs_all = qk_raw if raw else qk_all
            chunks = ((0, min(512, w)),) + (((512, w - 512),) if w > 512 else ())
            e_sb = e_sb_pool.tile([128, MT, QCH], F32R, name="e_sb")
            s_ps = psum_s.tile([128, QCH], F32, name="s_ps")

            def sum_mm(t):
                # s[q] (broadcast across all partitions) via all-ones lhsT,
                # accumulated over the 18 m-tiles
                for off, size in chunks:
                    nc.tensor.matmul(
                        s_ps[:, off : off + size],
                        lhsT=ones_r[:],
                        rhs=e_sb[:, t, off : off + size],
                        start=(t == 0),
                        stop=(t == MT - 1),
                    )

            # z = mk4.T @ qk -> exp(z + bias) -> SBUF; the sum-matmul for
            # tile t-1 is interleaved after the main matmul of tile t so the
            # in-order TensorE pipeline never waits a full exp phase.
            for t in range(MT):
                e_ps = psum_e.tile([128, QCH], F32, name="e_ps")
                for off, size in chunks:
                    nc.tensor.matmul(
                        e_ps[:, off : off + size],
                        lhsT=lhs_all[b * CK : (b + 1) * CK, t * 128 : (t + 1) * 128],
                        rhs=rhs_all[b * CK : (b + 1) * CK, q0 + off : q0 + off + size],
                        start=True,
                        stop=True,
                    )
                nc.scalar.activation(
                    e_sb[:, t, 0:w],
                    e_ps[:, 0:w],
                    AF.Exp,
                    bias=neg_a8[:, b * MT + t : b * MT + t + 1],
                    scale=0.25 if raw else 1.0,
                )
                if t >= 1:
                    sum_mm(t - 1)
            sum_mm(MT - 1)
            r_sb = r_pool.tile([128, QCH], F32, name="r_sb")
            nc.vector.reciprocal_approx_fast(r_sb[:, 0:w], s_ps[:, 0:w])
            # normalize and store. For the narrow warmup chunk the per-op DVE
            # overhead would starve the store DMA, so two m-tiles share one
            # multiply + one DMA (r is replicated once to line the APs up).
            if w <= 384:
                r2 = r_pool.tile([128, QCH], F32, name="r_sb")
                nc.vector.tensor_copy(r2[:, 0:w], r_sb[:, 0:w])
                nc.vector.tensor_copy(r2[:, w : 2 * w], r_sb[:, 0:w])
                r2v = r2[:, 0 : 2 * w].rearrange("p (u q) -> p u q", u=2)
                for t2 in range(MT // 2):
                    o_sb = o_pool.tile([128, QCH], F32, name="o_sb")
                    ov = o_sb[:, 0 : 2 * w].rearrange("p (u q) -> p u q", u=2)
                    nc.vector.tensor_mul(
                        ov, e_sb[:, 2 * t2 : 2 * t2 + 2, 0:w].bitcast(F32), r2v
                    )
                    dest = out_ext[
                        b, 2 * t2 * 128 : (2 * t2 + 2) * 128, q0 : q0 + w
                    ].rearrange("(u p) q -> p u q", u=2)
                    nc.sync.dma_start(dest, ov)
            else:
                for t in range(MT):
                    o_sb = o_pool.tile([128, QCH], F32, name="o_sb")
                    nc.vector.tensor_mul(
                        o_sb[:, 0:w], e_sb[:, t, 0:w].bitcast(F32), r_sb[:, 0:w]
                    )
                    nc.sync.dma_start(
                        out_ext[b, t * 128 : (t + 1) * 128, q0 : q0 + w], o_sb[:, 0:w]
                    )

        # b=0's q range ramps 256/512/768/768 so the store pipeline starts
        # early and deepens smoothly (the warmup region is DMA-starved anyway)
        jobs = [(0, 0, 256, True), (0, 256, 384, False), (0, 640, 384, False), (0, 1024, 512, False)]
        jobs += [(0, 1536, QCH, False)]
        jobs += [(1, q0, QCH, False) for q0 in range(0, HW, QCH)]
        for b, q0, w, raw in jobs:
            if raw:
                emit_warmup_chunk(q0, w)
            else:
                emit_chunk(b, q0, w, raw)


_CACHE = {}


def _get_compiled(niter: int = 1):
    """Build+compile the per-core graph. niter>1 repeats the whole body
    (same inputs/outputs) for wall-clock-difference timing in test.py."""
    key = ("nc", niter)
    if key not in _CACHE:
        nc = bacc.Bacc("TRN2", target_bir_lowering=False, debug=False)
        mk_ext = nc.dram_tensor("mk", [BL, CK, HW], F32, kind="ExternalInput").ap()
        qk_ext = nc.dram_tensor("qk", [BL, CK, HW], F32, kind="ExternalInput").ap()
        out_ext = nc.dram_tensor("out", [BL, HW, HW], F32, kind="ExternalOutput").ap()
        with tile.TileContext(nc) as tc:
            for _ in range(niter):
                _build_kernel(tc, out_ext, mk_ext, qk_ext)
        nc.compile()
        _CACHE[key] = nc
    return _CACHE[key]


def run_spmd(mk: np.ndarray, qk: np.ndarray, niter: int = 1) -> np.ndarray:
    mk = np.ascontiguousarray(np.asarray(mk, dtype=np.float32).reshape(B, CK, HW))
    qk = np.ascontiguousarray(np.asarray(qk, dtype=np.float32).reshape(B, CK, HW))
    nc = _get_compiled(niter)
    in_maps = [
        {"mk": mk[c * BL : (c + 1) * BL], "qk": qk[c * BL : (c + 1) * BL]}
        for c in range(NCORES)
    ]
    res = run_bass_kernel_spmd(nc, in_maps, core_ids=list(range(NCORES)))
    out = np.concatenate([res.results[c]["out"] for c in range(NCORES)], axis=0)
    return out.reshape(B, HW, HW)


def kernel(mk: np.ndarray, qk: np.ndarray) -> np.ndarray:
    return run_spmd(mk, qk, niter=1)



# revision 2
# speedup vs baseline: 1.0081x; 1.0081x over previous
"""Trainium2 Bass kernel for nn_AttentionMemory (sparse_attention).

reference:
    mkf = mk.reshape(B, CK, HW); qkf = qk.reshape(B, CK, HW)
    affinity[b, m, q] = (-|mk_m|^2 + 2 mk_m.qk_q - |qk_q|^2) / sqrt(CK)
    out = softmax(affinity, axis=m)

Math: softmax over m drops any term constant in m, so -|qk_q|^2 vanishes:
    out[b, :, q] = softmax_m( (2 mk_m.qk_q - |mk_m|^2) / 8 )
The per-m bias -|mk_m|^2 is folded into the matmul as an augmented K row:
    lhsT = [mk ; -|mk_m|^2/2] (65 x m), rhs = [qk ; 1] (65 x q)
    z[m, q] = mk_m.qk_q - a_m/2        exp arg = z/4 (ACT scale operand)
so every Activation is bias-free and m-tiles can share one instruction.

Distribution: data parallel over B=16 -> 2 batches/core on 8 cores.

Per-core pipeline, per (batch, 512-wide q chunk):
  - 18 m-tile matmuls (fp32r, K=65) -> PSUM pairs [128, 2*512]
  - one Exp per PAIR of tiles (scale=0.25) -> e_sb bf16 [128, 18, 512]
  - column sums via all-ones bf16 lhsT matmuls accumulated over tiles
  - reciprocal -> r; bf16 16-bit multiplies e*r -> f16 out tiles
  - one store DMA per SIX m-tiles (contiguous 6*128 rows of out)
"""

import numpy as np
from contextlib import ExitStack

import concourse.bass as bass
import concourse.tile as tile
from concourse import bacc, mybir

B, CK, H, W = 16, 64, 36, 64
HW = H * W                 # 2304
NCORES = 8
BL = B // NCORES           # 2 batches per core
MT = HW // 128             # 18 m-tiles
KA = CK + 1                # augmented K (bias row)
QCH = 512                  # q chunk = one PSUM bank
F32 = mybir.dt.float32
F32R = mybir.dt.float32r
BF16 = mybir.dt.bfloat16
F16 = mybir.dt.float16
AF = mybir.ActivationFunctionType
ALU = mybir.AluOpType

CHUNKS = [(0, 512), (512, 512), (1024, 512), (1536, 512), (2048, 256)]
CHUNKS_B0 = [(0, 256), (256, 512), (768, 512), (1280, 512), (1792, 512)]
# b1 tapers: the final chunks shrink so the end-of-kernel store drain is short
CHUNKS_B1 = [(0, 512), (512, 512), (1024, 512), (1536, 384), (1920, 256), (2176, 128)]


def _build_kernel(tc: tile.TileContext, out_ext, mk_ext, qk_ext):
    nc = tc.nc
    with ExitStack() as ctx:
        singles = ctx.enter_context(tc.tile_pool(name="singles", bufs=1))
        mk_aug = [singles.tile([KA, HW], F32R, name=f"mk_aug{b}") for b in range(BL)]
        qk_aug = [singles.tile([KA, HW], F32R, name=f"qk_aug{b}") for b in range(BL)]
        ones_b = singles.tile([128, 128], BF16, name="ones_b")
        # selector stationary: ones in column CK=64 only, so the bias-row
        # matmul writes its sums straight onto partition 64 of PSUM
        sel = singles.tile([CK, 128], F32R, name="sel")
        sel_f32 = singles.tile([CK, 128], F32, name="sel_f32")
        ones_row = singles.tile([1, HW], F32, name="ones_row")
        dummy = singles.tile([1, 1], F32, name="dummy")

        prep = ctx.enter_context(tc.tile_pool(name="prep", bufs=1))
        e_pool = ctx.enter_context(tc.tile_pool(name="e_sb", bufs=2))
        o_pool = ctx.enter_context(tc.tile_pool(name="o_sb", bufs=6))
        r_pool = ctx.enter_context(tc.tile_pool(name="r", bufs=3))
        psum_e = ctx.enter_context(tc.tile_pool(name="psum_e", bufs=2, space="PSUM"))
        psum_s = ctx.enter_context(tc.tile_pool(name="psum_s", bufs=1, space="PSUM"))
        psum_a = ctx.enter_context(tc.tile_pool(name="psum_a", bufs=1, space="PSUM"))

        mk_flat = mk_ext.rearrange("b c n -> (b c) n")
        qk_flat = qk_ext.rearrange("b c n -> (b c) n")

        # ---- prep: input DMAs + bias row (-|mk_m|^2/2) via matmul ----
        # per-512-column pieces so the first chunk's matmuls start ~2.5 us in
        nc.vector.memset(sel_f32[:], 0.0)
        nc.vector.memset(sel_f32[:, CK : CK + 1], 1.0)
        nc.vector.tensor_copy(sel[:], sel_f32[:])
        nc.gpsimd.memset(ones_row[:], 1.0)
        nc.gpsimd.memset(ones_b[:], 1.0)
        sq2 = [prep.tile([CK, HW], F32R, name=f"sq2_{b}") for b in range(BL)]
        a_ps = psum_a.tile([128, QCH], F32, name="a_ps")

        def emit_arow(b, c0, cw):
            # the single lhsT column is placed at partition CK=64, so the
            # bias row lands on the partition mk_aug row 64 lives on
            nc.tensor.matmul(
                a_ps[:, 0:cw],
                lhsT=sel[:],
                rhs=sq2[b][:, c0 : c0 + cw],
                start=True,
                stop=True,
            )
            nc.vector.tensor_copy(
                mk_aug[b][CK : CK + 1, c0 : c0 + cw], a_ps[CK : CK + 1, 0:cw]
            )

        def emit_prep(b, split):
            mkf = mk_flat[b * CK : (b + 1) * CK, :]
            qkf = qk_flat[b * CK : (b + 1) * CK, :]
            if split:
                nc.gpsimd.dma_start(qk_aug[b][0:CK, 0:256], qkf[:, 0:256])
            else:
                nc.sync.dma_start(qk_aug[b][0:CK, :], qkf[:])
            nc.gpsimd.tensor_copy(qk_aug[b][CK : CK + 1, :], ones_row[:])
            first = True
            for c0, cw in CHUNKS if split else [(0, HW)]:
                nc.sync.dma_start(
                    mk_aug[b][0:CK, c0 : c0 + cw], mkf[:, c0 : c0 + cw]
                )
                if first and b == 0:
                    # tiny exp so the ACT table set loads during the DMAs
                    nc.scalar.activation(
                        dummy[:], ones_row[0:1, 0:1], AF.Exp
                    )
                first = False
                nc.vector.scalar_tensor_tensor(
                    out=sq2[b][:, c0 : c0 + cw],
                    in0=mk_aug[b][0:CK, c0 : c0 + cw].bitcast(F32),
                    scalar=-0.5,
                    in1=mk_aug[b][0:CK, c0 : c0 + cw].bitcast(F32),
                    op0=ALU.mult,
                    op1=ALU.mult,
                )
            if split:
                emit_arow(b, *CHUNKS[0])
                nc.sync.dma_start(qk_aug[b][0:CK, 256:], qkf[:, 256:])
            else:
                for c0, cw in CHUNKS:
                    emit_arow(b, c0, cw)

        # ---- main: one softmax pipeline per (batch, q-chunk) ----
        _dma_rr = [0]

        def emit_chunk(b, q0, w, pair_hook=None, last=False):
            e_sb = e_pool.tile([128, MT, QCH], BF16, name="e_sb")
            s_ps = psum_s.tile([128, QCH], F32, name="s_ps")

            def sum_mm(t):
                # s[q] broadcast to all partitions via all-ones bf16 lhsT,
                # accumulated over the 18 m-tiles
                nc.tensor.matmul(
                    s_ps[:, 0:w],
                    lhsT=ones_b[:],
                    rhs=e_sb[:, t, 0:w],
                    start=(t == 0),
                    stop=(t == MT - 1),
                )

            G = 3
            for tp in range(MT // G):
                e_ps = psum_e.tile([128, G * QCH], F32, name="e_ps")
                ev = e_ps[:].rearrange("p (u q) -> p u q", u=G)
                for u in range(G):
                    t = G * tp + u
                    nc.tensor.matmul(
                        e_ps[:, u * QCH : u * QCH + w],
                        lhsT=mk_aug[b][:, t * 128 : (t + 1) * 128],
                        rhs=qk_aug[b][:, q0 : q0 + w],
                        start=True,
                        stop=True,
                    )
                nc.scalar.activation(
                    e_sb[:, G * tp : G * tp + G, 0:w], ev[:, :, 0:w], AF.Exp,
                    scale=0.25,
                )
                if pair_hook is not None:
                    pair_hook(tp)
                if tp >= 1:
                    for u in range(G):
                        sum_mm(G * (tp - 1) + u)
            for u in range(G):
                sum_mm(MT - G + u)

            r_sb = r_pool.tile([128, QCH], F32, name="r_sb")
            nc.vector.reciprocal_approx_fast(r_sb[:, 0:w], s_ps[:, 0:w])
            rb2 = r_pool.tile([128, 2, QCH], BF16, name="rb2")
            for u in range(2):
                nc.vector.tensor_copy(rb2[:, u, 0:w], r_sb[:, 0:w])
            for g in range(MT // 6):
                o_sb = o_pool.tile([128, 6, QCH], F16, name="o_sb")
                for pp in range(3):
                    t = 6 * g + 2 * pp
                    nc.vector.tensor_mul(
                        o_sb[:, 2 * pp : 2 * pp + 2, 0:w],
                        e_sb[:, t : t + 2, 0:w],
                        rb2[:, :, 0:w],
                    )
                dest = out_ext[
                    b, 6 * g * 128 : (6 * g + 6) * 128, q0 : q0 + w
                ].rearrange("(u p) q -> p u q", u=6)
                if last:
                    eng = (nc.sync, nc.gpsimd, nc.scalar)[g % 3]
                else:
                    eng = (nc.sync, nc.gpsimd)[_dma_rr[0] % 2]
                    _dma_rr[0] += 1
                eng.dma_start(dest, o_sb[:, :, 0:w])

        emit_prep(0, split=True)

        def chunk0_hook(tp):
            # bias-row piece k feeds tiles 4k..; emit piece tp+1 right after
            # triple tp (triple tp+1 reads rows < (3tp+6)*128 <= (tp+2)*512)
            if tp < 4:
                emit_arow(0, *CHUNKS[tp + 1])

        for i, (q0, w) in enumerate(CHUNKS_B0):
            emit_chunk(0, q0, w, pair_hook=chunk0_hook if i == 0 else None)
            if i == 0:
                emit_prep(1, split=False)
        for i, (q0, w) in enumerate(CHUNKS_B1):
            emit_chunk(1, q0, w, last=(i >= len(CHUNKS_B1) - 2))


_CACHE = {}


def _get_compiled(niter: int = 1):
    """Build+compile the per-core graph. niter>1 wraps the body in a For_i
    hardware loop (identical I/O each iteration) for differential timing."""
    key = ("nc", niter)
    if key not in _CACHE:
        nc = bacc.Bacc("TRN2", target_bir_lowering=False, debug=False)
        mk_ext = nc.dram_tensor("mk", [BL, CK, HW], F32R, kind="ExternalInput").ap()
        qk_ext = nc.dram_tensor("qk", [BL, CK, HW], F32R, kind="ExternalInput").ap()
        out_ext = nc.dram_tensor("out", [BL, HW, HW], F16, kind="ExternalOutput").ap()
        with tile.TileContext(nc) as tc:
            if niter == 1:
                _build_kernel(tc, out_ext, mk_ext, qk_ext)
            else:
                with tc.For_i(0, niter, 1):
                    _build_kernel(tc, out_ext, mk_ext, qk_ext)
        nc.compile()
        _CACHE[key] = nc
    return _CACHE[key]


class _CachedRunner:
    """Compile/upload the executable once; reuse the jitted callable for
    every subsequent call (a fresh jax.jit per call re-lowers and re-loads
    the NEFF-embedding executable through the axon tunnel each time)."""

    def __init__(self, nc, n_cores: int):
        import jax
        from jax.sharding import Mesh, PartitionSpec, NamedSharding
        from jax.experimental.shard_map import shard_map
        from concourse import bass2jax

        bass2jax.install_neuronx_cc_hook()
        self.n_cores = n_cores
        partition_name = (
            nc.partition_id_tensor.name if nc.partition_id_tensor else None
        )
        in_names, out_names, out_avals, zero_outs = [], [], [], []
        for alloc in nc.m.functions[0].allocations:
            if not isinstance(alloc, mybir.MemoryLocationSet):
                continue
            name = alloc.memorylocations[0].name
            if alloc.kind == "ExternalInput":
                if name != partition_name:
                    in_names.append(name)
            elif alloc.kind == "ExternalOutput":
                out_names.append(name)
                shape = tuple(alloc.tensor_shape)
                dtype = mybir.dt.np(alloc.dtype)
                out_avals.append(jax.core.ShapedArray(shape, dtype))
                zero_outs.append(np.zeros(shape, dtype))
        n_params = len(in_names)
        in_names = in_names + out_names
        if partition_name is not None:
            in_names.append(partition_name)
        self.in_names, self.out_names = in_names, out_names
        self.n_params, self.out_avals = n_params, out_avals

        def _body(*args):
            operands = list(args)
            if partition_name is not None:
                operands.append(bass2jax.partition_id_tensor())
            return tuple(
                bass2jax._bass_exec_p.bind(
                    *operands,
                    out_avals=tuple(out_avals),
                    in_names=tuple(in_names),
                    out_names=tuple(out_names),
                    lowering_input_output_aliases=(),
                    sim_require_finite=True,
                    sim_require_nnan=True,
                    nc=nc,
                )
            )

        P = PartitionSpec
        mesh = Mesh(np.asarray(jax.devices()[:n_cores]), ("core",))
        self.fn = jax.jit(
            shard_map(
                _body,
                mesh=mesh,
                in_specs=(P("core"),) * (n_params + len(out_names)),
                out_specs=(P("core"),) * len(out_names),
                check_rep=False,
            ),
            keep_unused=True,
        )
        sharding = NamedSharding(mesh, P("core"))
        self.zeros_dev = [
            jax.device_put(
                np.zeros((n_cores * z.shape[0], *z.shape[1:]), z.dtype), sharding
            )
            for z in zero_outs
        ]

    def __call__(self, in_maps):
        concat_in = [
            np.concatenate([np.asarray(m[name]) for m in in_maps], axis=0)
            for name in self.in_names[: self.n_params]
        ]
        out_arrs = self.fn(*concat_in, *self.zeros_dev)
        # materialize each output ONCE (np.asarray on a sharded array
        # re-gathers the full array every call)
        host = [
            np.asarray(a).reshape(self.n_cores, *self.out_avals[i].shape)
            for i, a in enumerate(out_arrs)
        ]
        return [
            {name: host[i][c] for i, name in enumerate(self.out_names)}
            for c in range(self.n_cores)
        ]


def _get_runner(niter: int = 1) -> "_CachedRunner":
    key = ("runner", niter)
    if key not in _CACHE:
        _CACHE[key] = _CachedRunner(_get_compiled(niter), NCORES)
    return _CACHE[key]


def run_spmd(mk: np.ndarray, qk: np.ndarray, niter: int = 1) -> np.ndarray:
    mk = np.ascontiguousarray(np.asarray(mk, dtype=np.float32).reshape(B, CK, HW))
    qk = np.ascontiguousarray(np.asarray(qk, dtype=np.float32).reshape(B, CK, HW))
    in_maps = [
        {"mk": mk[c * BL : (c + 1) * BL], "qk": qk[c * BL : (c + 1) * BL]}
        for c in range(NCORES)
    ]
    res = _get_runner(niter)(in_maps)
    out = np.concatenate([res[c]["out"] for c in range(NCORES)], axis=0)
    return out.reshape(B, HW, HW)


def kernel(mk: np.ndarray, qk: np.ndarray) -> np.ndarray:
    return run_spmd(mk, qk, niter=1).astype(np.float32)
